# revision 31
# baseline (speedup 1.0000x reference)
"""Trainium2 Bass kernel for ConvexLinearAttention (elu(x)+1 linear attention).

Full-input contract: kernel(**inputs) takes the unsharded tensors
(x [2,2048,1024], wq/wk/wv/wo [1024,1024], bq/bk/bv/bo [1024]) and returns the
full output [2,2048,1024].

Sharding (8 cores): data-parallel over batch (2) x head-group-parallel (4 groups
of 4 heads).  Each core projects only its 256-wide head slice, runs the
linearized attention (attended = qf @ (kf^T V) / (qf @ sum(kf)) -- an exact
refactoring of the dense normalized scores), and emits a partial output
projection.  The host sums the 4 head-group partials per batch (the
tensor-parallel unshard).

All matmul operands are bf16 (fp32 PSUM accumulate): bf16 stationary tiles get
fast-weight-load + pull-ahead so LDWEIGHTS hides under the previous matmul
(fp32r stationary serializes them), and bf16 halves every DMA transfer.  Every
dram tensor is host-pre-swizzled to a per-partition-contiguous layout that
matches its SBUF destination, so DMAs split into 128 multi-KB descriptors
instead of ~1k small ones.  The elu(x)+1 feature map uses exp(min(x,0)) ==
min(exp(x),1): scalar exp straight from PSUM, gpsimd min (SBUF-only engine),
one fused vector relu+add.  Attention is computed unnormalized (d = bden @ qf,
a = bkv @ qf back-to-back on the PE, software-pipelined one chunk ahead of the
output projection) and normalized on the vector engine (att = a *
reciprocal_approx_fast(d)); the max(d, EPS) clamp is dropped because qf,kf > 0
makes d >= O(1e4) mathematically.
"""

from contextlib import ExitStack

import ml_dtypes
import numpy as np

import concourse.bass as bass
import concourse.mybir as mybir
import concourse.tile as tile
from concourse import bacc, bass_utils

F32 = mybir.dt.float32
BF16 = mybir.dt.bfloat16
FP8 = mybir.dt.float8e4
AF = mybir.ActivationFunctionType
ALU = mybir.AluOpType
DR = mybir.MatmulPerfMode.DoubleRow

# fp8 (e4m3) K|V and Q projections with DoubleRow (2 fp8 weights per PE cell,
# 256-wide contraction per matmul).  Weights are host-scaled by WSCALE so
# N(0, 1/sqrt(D)) entries clear the e4m3 subnormal range; the scale rides
# along as kf' = 16 kf / qf' = 16 qf (relu commutes with positive scale, the
# exp path descales via the activation's scale input) and cancels in the
# attention normalization, with wo/16 absorbing the v-path factor exactly.
USE_FP8 = False  # measured: rel err 3.5e-2 (> 2e-2 gate) and no speedup
WSCALE = 16.0

D = 1024          # model dim
S = 2048          # sequence length
BATCH = 2
CSL = 256         # head-slice width per core (4 heads x 64)
NG = 2            # 128-wide c-groups per core
P = 128
NDT = D // P      # 8 d-tiles
NST = S // P      # 16 s-tiles
SC = 512          # s-chunk (matmul moving dim)
NSC = S // SC     # 4 s-chunks
STC = SC // P     # 4 s-tiles per chunk
NET = D // P      # 8 e-tiles

_CACHE: dict = {}


def install_ntff_hook_shim():
    """Provide ``antenv.axon_hooks`` when the image ships only the antenv stub.

    concourse.bass_utils imports it unconditionally on the axon trace path;
    without this shim trace=True (or BASS_TRACE=1) crashes.  Registers the real
    ctypes NTFF hook when the axon .so is present, else a None-returning stub
    so tracing degrades gracefully.
    """
    import os
    import sys
    import types

    if "antenv.axon_hooks" in sys.modules:
        return
    try:
        import antenv
        import antenv.axon_hooks  # noqa: F401
        return  # real module exists
    except ImportError:
        pass
    mod = types.ModuleType("antenv.axon_hooks")
    state: dict = {"h": None}
    mod.set_axon_ntff_profile_hook = lambda h: state.__setitem__("h", h)
    mod.get_axon_ntff_profile_hook = lambda: state.get("h")
    sys.modules["antenv.axon_hooks"] = mod
    antenv.axon_hooks = mod
    so_path = "/opt/axon/libaxon_pjrt.so"
    if os.path.exists(so_path):
        try:
            from trn_agent_boot.trn_boot import _ntff_profile_via_ctypes

            state["h"] = _ntff_profile_via_ctypes(so_path)
        except Exception:
            pass


def _build_kernel_body(ctx: ExitStack, tc: tile.TileContext, t, use_biases):
    nc = tc.nc
    fp8 = USE_FP8 and not use_biases
    IDT = FP8 if fp8 else BF16

    xT = t["xT"].ap().rearrange("p (sc do s) -> p sc do s", do=NDT, s=SC)
    wqT = t["wqT"].ap().rearrange("p (do c) -> p do c", do=NDT)
    wkvT = t["wkvT"].ap().rearrange("p (do c) -> p do c", do=NDT)
    woT = t["woT"].ap().rearrange("p (g e) -> p g e", g=NG)
    outT = t["outT"].ap().rearrange("p (sc e s) -> p sc e s", sc=NSC, e=NET)

    # ---- resident SBUF tensors (one pool, distinct tags = distinct slots) -
    const = ctx.enter_context(tc.tile_pool(name="const", bufs=1))

    def single(shape, name, dtype=F32):
        return const.tile(shape, dtype, name=name, tag=name)

    wkv_sb = single([P, NDT, 2 * CSL], "wkv_sb", IDT)
    wq_sb = single([P, NDT, CSL], "wq_sb", IDT)
    wo_sb = single([P, NG, D], "wo_sb", BF16)
    x_sb = single([P, NSC, NDT, SC], "x_sb", IDT)
    qf_sb = single([P, NG, S], "qf_sb", BF16)
    if use_biases:
        bk_rep = single([P, CSL], "bk_rep")
        bv_rep = single([P, CSL], "bv_rep")
        bq_sb = single([P, NG], "bq_sb")
        bo_sb = single([P, NET], "bo_sb")

    # DMA issue order = compute-need order, interleaving wkv d-tile pairs with
    # x chunk-0 d-tile pairs so the first KV matmuls can start ~1us after the
    # queues open; then Q weights, the rest of x, out weights / biases.
    for i in range(NDT // 2):
        dsl = slice(2 * i, 2 * i + 2)
        nc.sync.dma_start(out=wkv_sb[:, dsl, :], in_=wkvT[:, dsl, :])
        nc.sync.dma_start(out=x_sb[:, 0, dsl, :], in_=xT[:, 0, dsl, :])
    nc.sync.dma_start(out=wq_sb, in_=wqT)
    for sc in range(1, NSC):
        for i in range(NDT // 2):
            dsl = slice(2 * i, 2 * i + 2)
            nc.sync.dma_start(out=x_sb[:, sc, dsl, :], in_=xT[:, sc, dsl, :])
    nc.sync.dma_start(out=wo_sb, in_=woT)
    if use_biases:
        nc.gpsimd.dma_start(out=bk_rep, in_=t["bk1"].ap().partition_broadcast(P))
        nc.gpsimd.dma_start(out=bv_rep, in_=t["bv1"].ap().partition_broadcast(P))
        nc.sync.dma_start(out=bq_sb, in_=t["bq2"].ap())
        nc.sync.dma_start(out=bo_sb, in_=t["bo8"].ap())

    # bkv[g][c',c] = KV[c',c] for head(c')==head(c) else 0 (block diagonal)
    # bden[g][c',c] = ksum[c'] for head(c')==head(c) else 0
    bkv = [single([P, P], f"bkv{g}", BF16) for g in range(NG)]
    bden = [single([P, P], f"bden{g}", BF16) for g in range(NG)]
    for g in range(NG):
        nc.gpsimd.memset(bkv[g], 0.0)
        nc.gpsimd.memset(bden[g], 0.0)

    # PE warmup: the HAM clock gate holds the PE at 1.2 GHz until ~3.4us of
    # sustained activity.  Burn the initial DMA wait (~10us) on dummy matmuls
    # over zeroed tiles so the real matmuls start at 2.4 GHz.  36 N=512 MMs
    # end just before the first operands normally land.
    warm_sb = single([P, SC], "warm_sb", BF16)
    nc.gpsimd.memset(warm_sb, 0.0)
    with tc.tile_pool(name="ps_w", bufs=1, space="PSUM") as ps_w:
        w_ps = ps_w.tile([P, SC], F32, tag="w_ps")
        for _ in range(36):
            nc.tensor.matmul(w_ps, bkv[0], warm_sb, start=True, stop=True)

    # ---- phase A: K|V projection + feature map + KV/ksum accumulation,
    #      with Q projection interleaved per s-chunk -----------------------
    with tc.tile_pool(name="ps_kv", bufs=1, space="PSUM") as ps_kv, \
         tc.tile_pool(name="ps_a", bufs=3, space="PSUM") as ps_a, \
         tc.tile_pool(name="ps_q", bufs=3, space="PSUM") as ps_q, \
         tc.tile_pool(name="sb_a", bufs=4) as sb_a:
        kv_ps = [ps_kv.tile([P, CSL + 2], F32, name=f"kv_ps{g}", tag=f"kv{g}")
                 for g in range(NG)]

        # software pipeline: the KV/ksum accumulation for s-tile st issues
        # after s-tile st+1's projection matmuls, so the PE never waits on
        # the (vector) feature map of the tile it just projected.
        pending = None  # (st, kf, vaug)

        def flush_accum():
            nonlocal pending
            if pending is None:
                return
            pst, pkf, pvaug = pending
            for g in range(NG):
                nc.tensor.matmul(
                    kv_ps[g], pkf[:, g * P:(g + 1) * P], pvaug,
                    start=(pst == 0), stop=(pst == NST - 1))
            pending = None

        for sc in range(NSC):
            for sti in range(STC):
                st = sc * STC + sti
                ssl = slice(sti * P, (sti + 1) * P)
                # combined K|V projection: [s, 0:256]=K, [s, 256:512]=V
                kvp = ps_a.tile([P, 2 * CSL], F32, tag="kvp")
                if fp8:
                    for i in range(NDT // 2):
                        dsl = slice(2 * i, 2 * i + 2)
                        nc.tensor.matmul(
                            kvp, x_sb[:, sc, dsl, ssl], wkv_sb[:, dsl, :],
                            start=(i == 0), stop=(i == NDT // 2 - 1),
                            perf_mode=DR)
                else:
                    for dt in range(NDT):
                        nc.tensor.matmul(
                            kvp, x_sb[:, sc, dt, ssl], wkv_sb[:, dt, :],
                            start=(dt == 0), stop=(dt == NDT - 1))
                flush_accum()
                # kf = relu(K+bk) + exp(min(K+bk, 0)), with
                # exp(min(k,0)) == min(exp(k), 1) so scalar reads PSUM
                # directly and the min runs on the (SBUF-only) gpsimd.
                kf = sb_a.tile([P, CSL], BF16, tag="kf")
                e_k = sb_a.tile([P, CSL], BF16, tag="e_k")
                vaug = sb_a.tile([P, CSL + 2], BF16, tag="vaug")
                nc.gpsimd.memset(vaug[:, CSL:CSL + 2], 1.0)
                if use_biases:
                    kb = sb_a.tile([P, CSL], BF16, tag="kb")
                    nc.vector.tensor_tensor(kb, kvp[:, 0:CSL], bk_rep, ALU.add)
                    nc.scalar.activation(e_k, kb, AF.Exp)
                    nc.vector.tensor_scalar_min(e_k, e_k, 1.0)
                    nc.vector.scalar_tensor_tensor(
                        kf, kb, 0.0, e_k, op0=ALU.max, op1=ALU.add)
                    nc.vector.tensor_tensor(
                        vaug[:, 0:CSL], kvp[:, CSL:2 * CSL], bv_rep, ALU.add)
                else:
                    # fp8: kvp holds 16k|16v; kf' = 16 kf = relu(16k) +
                    # 16*min(exp(k),1) -- the scale cancels downstream.
                    if fp8:
                        nc.scalar.activation(
                            e_k, kvp[:, 0:CSL], AF.Exp, scale=1.0 / WSCALE)
                        nc.vector.tensor_scalar(
                            e_k, e_k, 1.0, WSCALE, op0=ALU.min, op1=ALU.mult)
                    else:
                        nc.scalar.activation(e_k, kvp[:, 0:CSL], AF.Exp)
                        nc.vector.tensor_scalar_min(e_k, e_k, 1.0)
                    nc.vector.scalar_tensor_tensor(
                        kf, kvp[:, 0:CSL], 0.0, e_k, op0=ALU.max, op1=ALU.add)
                    nc.scalar.copy(out=vaug[:, 0:CSL], in_=kvp[:, CSL:2 * CSL])
                pending = (st, kf, vaug)

            # After the last KV accumulation, build the masked KV/ksum
            # stationary tiles (split scalar/vector) while the PE runs the
            # last chunk's Q projection.
            if sc == NSC - 1:
                flush_accum()
                for g in range(NG):
                    for hb in range(2):
                        hsl = slice(hb * 64, (hb + 1) * 64)
                        csl2 = slice(g * P + hb * 64, g * P + (hb + 1) * 64)
                        nc.vector.tensor_copy(
                            out=bkv[g][hsl, hsl], in_=kv_ps[g][hsl, csl2])
                        nc.scalar.copy(
                            out=bden[g][hsl, hsl],
                            in_=kv_ps[g][hsl, CSL:CSL + 1].to_broadcast((64, 64)))

            # Q projection for this chunk (x already in SBUF; fills PE
            # while DMA streams the next chunk)
            csl = slice(sc * SC, (sc + 1) * SC)
            for g in range(NG):
                q_ps = ps_q.tile([P, SC], F32, tag="q_ps")
                if fp8:
                    for i in range(NDT // 2):
                        dsl = slice(2 * i, 2 * i + 2)
                        nc.tensor.matmul(
                            q_ps, wq_sb[:, dsl, g * P:(g + 1) * P],
                            x_sb[:, sc, dsl, :],
                            start=(i == 0), stop=(i == NDT // 2 - 1),
                            perf_mode=DR)
                else:
                    for dt in range(NDT):
                        nc.tensor.matmul(
                            q_ps, wq_sb[:, dt, g * P:(g + 1) * P],
                            x_sb[:, sc, dt, :],
                            start=(dt == 0), stop=(dt == NDT - 1))
                e_q = sb_a.tile([P, SC], BF16, tag="e_q")
                if use_biases:
                    nc.vector.tensor_scalar(
                        e_q, q_ps, bq_sb[:, g:g + 1], 0.0,
                        op0=ALU.add, op1=ALU.min)
                    nc.scalar.activation(e_q, e_q, AF.Exp)
                    rq = sb_a.tile([P, SC], BF16, tag="rq")
                    nc.scalar.activation(
                        rq, q_ps, AF.Relu, bias=bq_sb[:, g:g + 1])
                    nc.vector.tensor_tensor(
                        qf_sb[:, g, csl], rq, e_q, ALU.add)
                else:
                    if fp8:
                        nc.scalar.activation(
                            e_q, q_ps, AF.Exp, scale=1.0 / WSCALE)
                        nc.vector.tensor_scalar(
                            e_q, e_q, 1.0, WSCALE, op0=ALU.min, op1=ALU.mult)
                    else:
                        nc.scalar.activation(e_q, q_ps, AF.Exp)
                        nc.vector.tensor_scalar_min(e_q, e_q, 1.0)
                    nc.vector.scalar_tensor_tensor(
                        qf_sb[:, g, csl], q_ps, 0.0, e_q,
                        op0=ALU.max, op1=ALU.add)
            flush_accum()

    # ---- phase B: attention + output projection per s-chunk --------------
    # d = bden @ qf and a = bkv @ qf issue back-to-back on the PE one chunk
    # AHEAD of the output projection, so the (vector) normalization
    # att = a * 1/d of chunk sc overlaps the PE's out-proj of chunk sc-1.
    with tc.tile_pool(name="ps_b", bufs=1, space="PSUM") as ps_b, \
         tc.tile_pool(name="ps_o", bufs=3, space="PSUM") as ps_o, \
         tc.tile_pool(name="sb_b", bufs=3) as sb_b, \
         tc.tile_pool(name="sb_o", bufs=2) as sb_o:

        def attend(sc):
            csl = slice(sc * SC, (sc + 1) * SC)
            att = []
            for g in range(NG):
                d_ps = ps_b.tile([P, SC], F32, tag=f"d_ps{g}")
                nc.tensor.matmul(d_ps, bden[g], qf_sb[:, g, csl],
                                 start=True, stop=True)
                a_ps = ps_b.tile([P, SC], F32, tag=f"a_ps{g}")
                nc.tensor.matmul(a_ps, bkv[g], qf_sb[:, g, csl],
                                 start=True, stop=True)
                rden = sb_b.tile([P, SC], F32, tag=f"rden{g}")
                nc.vector.reciprocal_approx_fast(out=rden, in_=d_ps)
                a_sb = sb_b.tile([P, SC], BF16, tag=f"att{g}")
                nc.vector.tensor_tensor(a_sb, a_ps, rden, ALU.mult)
                att.append(a_sb)
            return att

        att = attend(0)
        for sc in range(NSC):
            att_next = attend(sc + 1) if sc + 1 < NSC else None
            o_cat = sb_o.tile([P, NET, SC], BF16, tag="o_cat")
            for et in range(NET):
                esl = slice(et * P, (et + 1) * P)
                o_ps = ps_o.tile([P, SC], F32, tag="o_ps")
                for g in range(NG):
                    nc.tensor.matmul(o_ps, wo_sb[:, g, esl], att[g],
                                     start=(g == 0), stop=(g == NG - 1))
                if use_biases:
                    if et % 2 == 0:
                        nc.vector.tensor_scalar(
                            o_cat[:, et, :], o_ps, bo_sb[:, et:et + 1], None,
                            op0=ALU.add)
                    else:
                        nc.scalar.activation(
                            o_cat[:, et, :], o_ps, AF.Identity,
                            bias=bo_sb[:, et:et + 1])
                else:
                    if et % 2 == 0:
                        nc.vector.tensor_copy(out=o_cat[:, et, :], in_=o_ps)
                    else:
                        nc.scalar.copy(out=o_cat[:, et, :], in_=o_ps)
                if sc == NSC - 1:
                    # per-e-tile drain on the last chunk shortens the tail
                    nc.sync.dma_start(
                        out=outT[:, sc, et:et + 1, :],
                        in_=o_cat[:, et:et + 1, :])
                elif et % 2 == 1:
                    h = et // 2
                    nc.sync.dma_start(
                        out=outT[:, sc, 2 * h:2 * h + 2, :],
                        in_=o_cat[:, 2 * h:2 * h + 2, :])
            att = att_next


def build_nc(use_biases):
    nc = bacc.Bacc("TRN2", target_bir_lowering=False, debug=False)
    idt = FP8 if (USE_FP8 and not use_biases) else BF16
    t = {}
    t["xT"] = nc.dram_tensor("xT", [P, D * S // P], idt, kind="ExternalInput")
    t["wqT"] = nc.dram_tensor("wqT", [P, D * CSL // P], idt, kind="ExternalInput")
    t["wkvT"] = nc.dram_tensor(
        "wkvT", [P, D * 2 * CSL // P], idt, kind="ExternalInput")
    t["woT"] = nc.dram_tensor("woT", [P, CSL * D // P], BF16, kind="ExternalInput")
    if use_biases:
        t["bq2"] = nc.dram_tensor("bq2", [P, NG], F32, kind="ExternalInput")
        t["bk1"] = nc.dram_tensor("bk1", [CSL], F32, kind="ExternalInput")
        t["bv1"] = nc.dram_tensor("bv1", [CSL], F32, kind="ExternalInput")
        t["bo8"] = nc.dram_tensor("bo8", [P, NET], F32, kind="ExternalInput")
    t["outT"] = nc.dram_tensor("outT", [P, D * S // P], BF16, kind="ExternalOutput")

    with tile.TileContext(nc) as tc:
        with ExitStack() as ctx:
            _build_kernel_body(ctx, tc, t, use_biases)
    nc.compile()
    return nc


def _get_nc(use_biases):
    key = ("nc", use_biases)
    if key not in _CACHE:
        _CACHE[key] = build_nc(use_biases)
    return _CACHE[key]


def make_in_maps(x, wq, bq, wk, bk, wv, bv, wo, bo, use_biases=None):
    """Shard the full inputs into the 8 per-core input maps."""
    f = lambda a: np.asarray(a, dtype=np.float32)
    x, wq, bq, wk, bk = f(x), f(wq), f(bq), f(wk), f(bk)
    wv, bv, wo, bo = f(wv), f(bv), f(wo), f(bo)
    bf = lambda a: np.ascontiguousarray(a).astype(ml_dtypes.bfloat16)
    if use_biases is None:
        use_biases = any(np.any(b) for b in (bq, bk, bv, bo))
    fp8 = USE_FP8 and not use_biases
    idt = ml_dtypes.float8_e4m3 if fp8 else ml_dtypes.bfloat16
    wsc = WSCALE if fp8 else 1.0

    # Pre-swizzle to per-partition-contiguous layouts: dram row p holds all
    # of partition p's data back-to-back, matching the SBUF destination.
    def swz(a):  # [(do p), f] -> [p, (do f)]
        dd, f_ = a.shape
        return np.ascontiguousarray(
            a.reshape(dd // P, P, f_).transpose(1, 0, 2).reshape(P, -1)
        ).astype(idt)

    def swz_x(a):  # [(do p), (sc s)] -> [p, (sc do s)]
        return np.ascontiguousarray(
            a.reshape(NDT, P, NSC, SC).transpose(1, 2, 0, 3).reshape(P, -1)
        ).astype(idt)

    in_maps = []
    for cid in range(8):
        b, hg = divmod(cid, 4)
        hs = slice(hg * CSL, (hg + 1) * CSL)
        m = {
            "xT": swz_x(x[b].T),
            "wqT": swz(wq[hs, :].T * wsc),
            "wkvT": swz(
                np.concatenate([wk[hs, :].T, wv[hs, :].T], axis=1) * wsc),
            "woT": bf(
                (wo[:, hs].T / wsc).reshape(NG, P, D)
                .transpose(1, 0, 2).reshape(P, -1)),
        }
        if use_biases:
            bo_in = bo if hg == 0 else np.zeros_like(bo)
            m["bq2"] = np.ascontiguousarray(bq[hs].reshape(NG, P).T)
            m["bk1"] = bk[hs].copy()
            m["bv1"] = bv[hs].copy()
            m["bo8"] = np.ascontiguousarray(bo_in.reshape(NET, P).T)
        in_maps.append(m)
    return in_maps, use_biases


def unshard(results):
    """Sum head-group partials per batch and undo the output swizzle."""
    out = np.zeros((BATCH, S, D), np.float32)
    for cid in range(8):
        b = cid // 4
        # outT[p, (sc e s)]: out[b, sc*SC+s, e*P+p] += outT[p, sc, e, s]
        arr = results[cid]["outT"].astype(np.float32)
        arr = arr.reshape(P, NSC, NET, SC).transpose(1, 3, 2, 0).reshape(S, D)
        out[b] += arr
    return out


def kernel(x, wq, bq, wk, bk, wv, bv, wo, bo):
    in_maps, use_biases = make_in_maps(x, wq, bq, wk, bk, wv, bv, wo, bo)
    nc = _get_nc(use_biases)
    res = bass_utils.run_bass_kernel_spmd(nc, in_maps, core_ids=list(range(8)))
    return unshard(res.results)


# revision 33
# speedup vs baseline: 1.0403x; 1.0403x over previous
"""Trainium2 Bass kernel for ConvexLinearAttention (elu(x)+1 linear attention).

Full-input contract: kernel(**inputs) takes the unsharded tensors
(x [2,2048,1024], wq/wk/wv/wo [1024,1024], bq/bk/bv/bo [1024]) and returns the
full output [2,2048,1024].

Sharding (8 cores): data-parallel over batch (2) x head-group-parallel (4 groups
of 4 heads).  Each core projects only its 256-wide head slice, runs the
linearized attention (attended = qf @ (kf^T V) / (qf @ sum(kf)) -- an exact
refactoring of the dense normalized scores), and emits a partial output
projection.  The host sums the 4 head-group partials per batch (the
tensor-parallel unshard).

All matmul operands are bf16 (fp32 PSUM accumulate): bf16 stationary tiles get
fast-weight-load + pull-ahead so LDWEIGHTS hides under the previous matmul
(fp32r stationary serializes them), and bf16 halves every DMA transfer.  Every
dram tensor is host-pre-swizzled to a per-partition-contiguous layout that
matches its SBUF destination, so DMAs split into 128 multi-KB descriptors
instead of ~1k small ones.  The elu(x)+1 feature map uses exp(min(x,0)) ==
min(exp(x),1): scalar exp straight from PSUM, gpsimd min (SBUF-only engine),
one fused vector relu+add.  Attention is computed unnormalized (d = bden @ qf,
a = bkv @ qf back-to-back on the PE, software-pipelined one chunk ahead of the
output projection) and normalized on the vector engine (att = a *
reciprocal_approx_fast(d)); the max(d, EPS) clamp is dropped because qf,kf > 0
makes d >= O(1e4) mathematically.
"""

from contextlib import ExitStack

import ml_dtypes
import numpy as np

import concourse.bass as bass
import concourse.mybir as mybir
import concourse.tile as tile
from concourse import bacc, bass_utils

F32 = mybir.dt.float32
BF16 = mybir.dt.bfloat16
FP8 = mybir.dt.float8e4
AF = mybir.ActivationFunctionType
ALU = mybir.AluOpType
DR = mybir.MatmulPerfMode.DoubleRow

# fp8 (e4m3) K|V and Q projections with DoubleRow (2 fp8 weights per PE cell,
# 256-wide contraction per matmul).  Weights are host-scaled by WSCALE so
# N(0, 1/sqrt(D)) entries clear the e4m3 subnormal range; the scale rides
# along as kf' = 16 kf / qf' = 16 qf (relu commutes with positive scale, the
# exp path descales via the activation's scale input) and cancels in the
# attention normalization, with wo/16 absorbing the v-path factor exactly.
USE_FP8 = False  # measured: rel err 3.5e-2 (> 2e-2 gate) and no speedup
WSCALE = 16.0

D = 1024          # model dim
S = 2048          # sequence length
BATCH = 2
CSL = 256         # head-slice width per core (4 heads x 64)
NG = 2            # 128-wide c-groups per core
P = 128
NDT = D // P      # 8 d-tiles
NST = S // P      # 16 s-tiles
SC = 512          # s-chunk (matmul moving dim)
NSC = S // SC     # 4 s-chunks
STC = SC // P     # 4 s-tiles per chunk
NET = D // P      # 8 e-tiles

_CACHE: dict = {}


def install_ntff_hook_shim():
    """Provide ``antenv.axon_hooks`` when the image ships only the antenv stub.

    concourse.bass_utils imports it unconditionally on the axon trace path;
    without this shim trace=True (or BASS_TRACE=1) crashes.  Registers the real
    ctypes NTFF hook when the axon .so is present, else a None-returning stub
    so tracing degrades gracefully.
    """
    import os
    import sys
    import types

    if "antenv.axon_hooks" in sys.modules:
        return
    try:
        import antenv
        import antenv.axon_hooks  # noqa: F401
        return  # real module exists
    except ImportError:
        pass
    mod = types.ModuleType("antenv.axon_hooks")
    state: dict = {"h": None}
    mod.set_axon_ntff_profile_hook = lambda h: state.__setitem__("h", h)
    mod.get_axon_ntff_profile_hook = lambda: state.get("h")
    sys.modules["antenv.axon_hooks"] = mod
    antenv.axon_hooks = mod
    so_path = "/opt/axon/libaxon_pjrt.so"
    if os.path.exists(so_path):
        try:
            from trn_agent_boot.trn_boot import _ntff_profile_via_ctypes

            state["h"] = _ntff_profile_via_ctypes(so_path)
        except Exception:
            pass


def _build_kernel_body(ctx: ExitStack, tc: tile.TileContext, t, use_biases):
    nc = tc.nc
    fp8 = USE_FP8 and not use_biases
    IDT = FP8 if fp8 else BF16

    xT = t["xT"].ap().rearrange("p (sc do s) -> p sc do s", do=NDT, s=SC)
    wqT = t["wqT"].ap().rearrange("p (do c) -> p do c", do=NDT)
    wkvT = t["wkvT"].ap().rearrange("p (do c) -> p do c", do=NDT)
    woT = t["woT"].ap().rearrange("p (g e) -> p g e", g=NG)
    outT = t["outT"].ap().rearrange("p (sc e s) -> p sc e s", sc=NSC, e=NET)

    # ---- resident SBUF tensors (one pool, distinct tags = distinct slots) -
    const = ctx.enter_context(tc.tile_pool(name="const", bufs=1))

    def single(shape, name, dtype=F32):
        return const.tile(shape, dtype, name=name, tag=name)

    wkv_sb = single([P, NDT, 2 * CSL], "wkv_sb", IDT)
    wq_sb = single([P, NDT, CSL], "wq_sb", IDT)
    wo_sb = single([P, NG, D], "wo_sb", BF16)
    x_sb = single([P, NSC, NDT, SC], "x_sb", IDT)
    qf_sb = single([P, NG, S], "qf_sb", BF16)
    if use_biases:
        bk_rep = single([P, CSL], "bk_rep")
        bv_rep = single([P, CSL], "bv_rep")
        bq_sb = single([P, NG], "bq_sb")
        bo_sb = single([P, NET], "bo_sb")

    # DMA issue order = compute-need order, interleaving wkv d-tile pairs with
    # x chunk-0 d-tile pairs so the first KV matmuls can start ~1us after the
    # queues open; then Q weights, the rest of x, out weights / biases.
    for i in range(NDT // 2):
        dsl = slice(2 * i, 2 * i + 2)
        nc.sync.dma_start(out=wkv_sb[:, dsl, :], in_=wkvT[:, dsl, :])
        nc.sync.dma_start(out=x_sb[:, 0, dsl, :], in_=xT[:, 0, dsl, :])
    nc.sync.dma_start(out=wq_sb, in_=wqT)
    for sc in range(1, NSC):
        for i in range(NDT // 2):
            dsl = slice(2 * i, 2 * i + 2)
            nc.sync.dma_start(out=x_sb[:, sc, dsl, :], in_=xT[:, sc, dsl, :])
    nc.sync.dma_start(out=wo_sb, in_=woT)
    if use_biases:
        nc.gpsimd.dma_start(out=bk_rep, in_=t["bk1"].ap().partition_broadcast(P))
        nc.gpsimd.dma_start(out=bv_rep, in_=t["bv1"].ap().partition_broadcast(P))
        nc.sync.dma_start(out=bq_sb, in_=t["bq2"].ap())
        nc.sync.dma_start(out=bo_sb, in_=t["bo8"].ap())

    # bkv[g][c',c] = KV[c',c] for head(c')==head(c) else 0 (block diagonal)
    # bden[g][c',c] = ksum[c'] for head(c')==head(c) else 0
    bkv = [single([P, P], f"bkv{g}", BF16) for g in range(NG)]
    bden = [single([P, P], f"bden{g}", BF16) for g in range(NG)]
    for g in range(NG):
        nc.gpsimd.memset(bkv[g], 0.0)
        nc.gpsimd.memset(bden[g], 0.0)

    # PE warmup: the HAM clock gate holds the PE at 1.2 GHz until ~3.4us of
    # sustained activity.  Burn the initial DMA wait on a few dummy matmuls
    # over zeroed tiles so the real matmuls start at 2.4 GHz.  8 N=512 MMs
    # cover one cold SHORT window and end before the first operands land
    # (a short PE idle after them does not re-throttle; only >3.4us does).
    warm_sb = single([P, SC], "warm_sb", BF16)
    nc.gpsimd.memset(warm_sb, 0.0)
    with tc.tile_pool(name="ps_w", bufs=1, space="PSUM") as ps_w:
        w_ps = ps_w.tile([P, SC], F32, tag="w_ps")
        for _ in range(8):
            nc.tensor.matmul(w_ps, bkv[0], warm_sb, start=True, stop=True)

    # ---- phase A: K|V projection + feature map + KV/ksum accumulation,
    #      with Q projection interleaved per s-chunk -----------------------
    with tc.tile_pool(name="ps_kv", bufs=1, space="PSUM") as ps_kv, \
         tc.tile_pool(name="ps_a", bufs=3, space="PSUM") as ps_a, \
         tc.tile_pool(name="ps_q", bufs=3, space="PSUM") as ps_q, \
         tc.tile_pool(name="sb_a", bufs=4) as sb_a:
        kv_ps = [ps_kv.tile([P, CSL + 2], F32, name=f"kv_ps{g}", tag=f"kv{g}")
                 for g in range(NG)]

        # software pipeline: the KV/ksum accumulation for s-tile st issues
        # after s-tile st+1's projection matmuls, so the PE never waits on
        # the (vector) feature map of the tile it just projected.
        pending = None  # (st, kf, vaug)

        def flush_accum():
            nonlocal pending
            if pending is None:
                return
            pst, pkf, pvaug = pending
            for g in range(NG):
                nc.tensor.matmul(
                    kv_ps[g], pkf[:, g * P:(g + 1) * P], pvaug,
                    start=(pst == 0), stop=(pst == NST - 1))
            pending = None

        for sc in range(NSC):
            for sti in range(STC):
                st = sc * STC + sti
                ssl = slice(sti * P, (sti + 1) * P)
                # combined K|V projection: [s, 0:256]=K, [s, 256:512]=V
                kvp = ps_a.tile([P, 2 * CSL], F32, tag="kvp")
                if fp8:
                    for i in range(NDT // 2):
                        dsl = slice(2 * i, 2 * i + 2)
                        nc.tensor.matmul(
                            kvp, x_sb[:, sc, dsl, ssl], wkv_sb[:, dsl, :],
                            start=(i == 0), stop=(i == NDT // 2 - 1),
                            perf_mode=DR)
                else:
                    for dt in range(NDT):
                        nc.tensor.matmul(
                            kvp, x_sb[:, sc, dt, ssl], wkv_sb[:, dt, :],
                            start=(dt == 0), stop=(dt == NDT - 1))
                flush_accum()
                # kf = relu(K+bk) + exp(min(K+bk, 0)), with
                # exp(min(k,0)) == min(exp(k), 1) so scalar reads PSUM
                # directly and the min runs on the (SBUF-only) gpsimd.
                kf = sb_a.tile([P, CSL], BF16, tag="kf")
                e_k = sb_a.tile([P, CSL], BF16, tag="e_k")
                vaug = sb_a.tile([P, CSL + 2], BF16, tag="vaug")
                nc.gpsimd.memset(vaug[:, CSL:CSL + 2], 1.0)
                if use_biases:
                    kb = sb_a.tile([P, CSL], BF16, tag="kb")
                    nc.vector.tensor_tensor(kb, kvp[:, 0:CSL], bk_rep, ALU.add)
                    nc.scalar.activation(e_k, kb, AF.Exp)
                    nc.vector.tensor_scalar_min(e_k, e_k, 1.0)
                    nc.vector.scalar_tensor_tensor(
                        kf, kb, 0.0, e_k, op0=ALU.max, op1=ALU.add)
                    nc.vector.tensor_tensor(
                        vaug[:, 0:CSL], kvp[:, CSL:2 * CSL], bv_rep, ALU.add)
                else:
                    # fp8: kvp holds 16k|16v; kf' = 16 kf = relu(16k) +
                    # 16*min(exp(k),1) -- the scale cancels downstream.
                    if fp8:
                        nc.scalar.activation(
                            e_k, kvp[:, 0:CSL], AF.Exp, scale=1.0 / WSCALE)
                        nc.vector.tensor_scalar(
                            e_k, e_k, 1.0, WSCALE, op0=ALU.min, op1=ALU.mult)
                    else:
                        nc.scalar.activation(e_k, kvp[:, 0:CSL], AF.Exp)
                        nc.vector.tensor_scalar_min(e_k, e_k, 1.0)
                    nc.vector.scalar_tensor_tensor(
                        kf, kvp[:, 0:CSL], 0.0, e_k, op0=ALU.max, op1=ALU.add)
                    nc.scalar.copy(out=vaug[:, 0:CSL], in_=kvp[:, CSL:2 * CSL])
                pending = (st, kf, vaug)

            # After the last KV accumulation, build the masked KV/ksum
            # stationary tiles (split scalar/vector) while the PE runs the
            # last chunk's Q projection.
            if sc == NSC - 1:
                flush_accum()
                for g in range(NG):
                    for hb in range(2):
                        hsl = slice(hb * 64, (hb + 1) * 64)
                        csl2 = slice(g * P + hb * 64, g * P + (hb + 1) * 64)
                        nc.vector.tensor_copy(
                            out=bkv[g][hsl, hsl], in_=kv_ps[g][hsl, csl2])
                        nc.scalar.copy(
                            out=bden[g][hsl, hsl],
                            in_=kv_ps[g][hsl, CSL:CSL + 1].to_broadcast((64, 64)))

            # Q projection for this chunk (x already in SBUF; fills PE
            # while DMA streams the next chunk)
            csl = slice(sc * SC, (sc + 1) * SC)
            for g in range(NG):
                q_ps = ps_q.tile([P, SC], F32, tag="q_ps")
                if fp8:
                    for i in range(NDT // 2):
                        dsl = slice(2 * i, 2 * i + 2)
                        nc.tensor.matmul(
                            q_ps, wq_sb[:, dsl, g * P:(g + 1) * P],
                            x_sb[:, sc, dsl, :],
                            start=(i == 0), stop=(i == NDT // 2 - 1),
                            perf_mode=DR)
                else:
                    for dt in range(NDT):
                        nc.tensor.matmul(
                            q_ps, wq_sb[:, dt, g * P:(g + 1) * P],
                            x_sb[:, sc, dt, :],
                            start=(dt == 0), stop=(dt == NDT - 1))
                e_q = sb_a.tile([P, SC], BF16, tag="e_q")
                if use_biases:
                    nc.vector.tensor_scalar(
                        e_q, q_ps, bq_sb[:, g:g + 1], 0.0,
                        op0=ALU.add, op1=ALU.min)
                    nc.scalar.activation(e_q, e_q, AF.Exp)
                    rq = sb_a.tile([P, SC], BF16, tag="rq")
                    nc.scalar.activation(
                        rq, q_ps, AF.Relu, bias=bq_sb[:, g:g + 1])
                    nc.vector.tensor_tensor(
                        qf_sb[:, g, csl], rq, e_q, ALU.add)
                else:
                    if fp8:
                        nc.scalar.activation(
                            e_q, q_ps, AF.Exp, scale=1.0 / WSCALE)
                        nc.vector.tensor_scalar(
                            e_q, e_q, 1.0, WSCALE, op0=ALU.min, op1=ALU.mult)
                    else:
                        nc.scalar.activation(e_q, q_ps, AF.Exp)
                        nc.vector.tensor_scalar_min(e_q, e_q, 1.0)
                    nc.vector.scalar_tensor_tensor(
                        qf_sb[:, g, csl], q_ps, 0.0, e_q,
                        op0=ALU.max, op1=ALU.add)
            flush_accum()

    # ---- phase B: attention + output projection per s-chunk --------------
    # d = bden @ qf and a = bkv @ qf issue back-to-back on the PE one chunk
    # AHEAD of the output projection, so the (vector) normalization
    # att = a * 1/d of chunk sc overlaps the PE's out-proj of chunk sc-1.
    with tc.tile_pool(name="ps_b", bufs=1, space="PSUM") as ps_b, \
         tc.tile_pool(name="ps_o", bufs=3, space="PSUM") as ps_o, \
         tc.tile_pool(name="sb_b", bufs=3) as sb_b, \
         tc.tile_pool(name="sb_o", bufs=2) as sb_o:

        def attend(sc):
            csl = slice(sc * SC, (sc + 1) * SC)
            att = []
            for g in range(NG):
                d_ps = ps_b.tile([P, SC], F32, tag=f"d_ps{g}")
                nc.tensor.matmul(d_ps, bden[g], qf_sb[:, g, csl],
                                 start=True, stop=True)
                a_ps = ps_b.tile([P, SC], F32, tag=f"a_ps{g}")
                nc.tensor.matmul(a_ps, bkv[g], qf_sb[:, g, csl],
                                 start=True, stop=True)
                rden = sb_b.tile([P, SC], F32, tag=f"rden{g}")
                nc.vector.reciprocal_approx_fast(out=rden, in_=d_ps)
                a_sb = sb_b.tile([P, SC], BF16, tag=f"att{g}")
                nc.vector.tensor_tensor(a_sb, a_ps, rden, ALU.mult)
                att.append(a_sb)
            return att

        att = attend(0)
        for sc in range(NSC):
            att_next = attend(sc + 1) if sc + 1 < NSC else None
            o_cat = sb_o.tile([P, NET, SC], BF16, tag="o_cat")
            for et in range(NET):
                esl = slice(et * P, (et + 1) * P)
                o_ps = ps_o.tile([P, SC], F32, tag="o_ps")
                for g in range(NG):
                    nc.tensor.matmul(o_ps, wo_sb[:, g, esl], att[g],
                                     start=(g == 0), stop=(g == NG - 1))
                if use_biases:
                    if et % 2 == 0:
                        nc.vector.tensor_scalar(
                            o_cat[:, et, :], o_ps, bo_sb[:, et:et + 1], None,
                            op0=ALU.add)
                    else:
                        nc.scalar.activation(
                            o_cat[:, et, :], o_ps, AF.Identity,
                            bias=bo_sb[:, et:et + 1])
                else:
                    if et % 2 == 0:
                        nc.vector.tensor_copy(out=o_cat[:, et, :], in_=o_ps)
                    else:
                        nc.scalar.copy(out=o_cat[:, et, :], in_=o_ps)
                if et % 2 == 1:
                    h = et // 2
                    nc.sync.dma_start(
                        out=outT[:, sc, 2 * h:2 * h + 2, :],
                        in_=o_cat[:, 2 * h:2 * h + 2, :])
            att = att_next


def build_nc(use_biases):
    nc = bacc.Bacc("TRN2", target_bir_lowering=False, debug=False)
    idt = FP8 if (USE_FP8 and not use_biases) else BF16
    t = {}
    t["xT"] = nc.dram_tensor("xT", [P, D * S // P], idt, kind="ExternalInput")
    t["wqT"] = nc.dram_tensor("wqT", [P, D * CSL // P], idt, kind="ExternalInput")
    t["wkvT"] = nc.dram_tensor(
        "wkvT", [P, D * 2 * CSL // P], idt, kind="ExternalInput")
    t["woT"] = nc.dram_tensor("woT", [P, CSL * D // P], BF16, kind="ExternalInput")
    if use_biases:
        t["bq2"] = nc.dram_tensor("bq2", [P, NG], F32, kind="ExternalInput")
        t["bk1"] = nc.dram_tensor("bk1", [CSL], F32, kind="ExternalInput")
        t["bv1"] = nc.dram_tensor("bv1", [CSL], F32, kind="ExternalInput")
        t["bo8"] = nc.dram_tensor("bo8", [P, NET], F32, kind="ExternalInput")
    t["outT"] = nc.dram_tensor("outT", [P, D * S // P], BF16, kind="ExternalOutput")

    with tile.TileContext(nc) as tc:
        with ExitStack() as ctx:
            _build_kernel_body(ctx, tc, t, use_biases)
    nc.compile()
    return nc


def _get_nc(use_biases):
    key = ("nc", use_biases)
    if key not in _CACHE:
        _CACHE[key] = build_nc(use_biases)
    return _CACHE[key]


def make_in_maps(x, wq, bq, wk, bk, wv, bv, wo, bo, use_biases=None):
    """Shard the full inputs into the 8 per-core input maps."""
    f = lambda a: np.asarray(a, dtype=np.float32)
    x, wq, bq, wk, bk = f(x), f(wq), f(bq), f(wk), f(bk)
    wv, bv, wo, bo = f(wv), f(bv), f(wo), f(bo)
    bf = lambda a: np.ascontiguousarray(a).astype(ml_dtypes.bfloat16)
    if use_biases is None:
        use_biases = any(np.any(b) for b in (bq, bk, bv, bo))
    fp8 = USE_FP8 and not use_biases
    idt = ml_dtypes.float8_e4m3 if fp8 else ml_dtypes.bfloat16
    wsc = WSCALE if fp8 else 1.0

    # Pre-swizzle to per-partition-contiguous layouts: dram row p holds all
    # of partition p's data back-to-back, matching the SBUF destination.
    def swz(a):  # [(do p), f] -> [p, (do f)]
        dd, f_ = a.shape
        return np.ascontiguousarray(
            a.reshape(dd // P, P, f_).transpose(1, 0, 2).reshape(P, -1)
        ).astype(idt)

    def swz_x(a):  # [(do p), (sc s)] -> [p, (sc do s)]
        return np.ascontiguousarray(
            a.reshape(NDT, P, NSC, SC).transpose(1, 2, 0, 3).reshape(P, -1)
        ).astype(idt)

    in_maps = []
    for cid in range(8):
        b, hg = divmod(cid, 4)
        hs = slice(hg * CSL, (hg + 1) * CSL)
        m = {
            "xT": swz_x(x[b].T),
            "wqT": swz(wq[hs, :].T * wsc),
            "wkvT": swz(
                np.concatenate([wk[hs, :].T, wv[hs, :].T], axis=1) * wsc),
            "woT": bf(
                (wo[:, hs].T / wsc).reshape(NG, P, D)
                .transpose(1, 0, 2).reshape(P, -1)),
        }
        if use_biases:
            bo_in = bo if hg == 0 else np.zeros_like(bo)
            m["bq2"] = np.ascontiguousarray(bq[hs].reshape(NG, P).T)
            m["bk1"] = bk[hs].copy()
            m["bv1"] = bv[hs].copy()
            m["bo8"] = np.ascontiguousarray(bo_in.reshape(NET, P).T)
        in_maps.append(m)
    return in_maps, use_biases


def unshard(results):
    """Sum head-group partials per batch and undo the output swizzle."""
    out = np.zeros((BATCH, S, D), np.float32)
    for cid in range(8):
        b = cid // 4
        # outT[p, (sc e s)]: out[b, sc*SC+s, e*P+p] += outT[p, sc, e, s]
        arr = results[cid]["outT"].astype(np.float32)
        arr = arr.reshape(P, NSC, NET, SC).transpose(1, 3, 2, 0).reshape(S, D)
        out[b] += arr
    return out


def kernel(x, wq, bq, wk, bk, wv, bv, wo, bo):
    in_maps, use_biases = make_in_maps(x, wq, bq, wk, bk, wv, bv, wo, bo)
    nc = _get_nc(use_biases)
    res = bass_utils.run_bass_kernel_spmd(nc, in_maps, core_ids=list(range(8)))
    return unshard(res.results)
